# revision 16
# baseline (speedup 1.0000x reference)
"""AnomalyAttention on 8 Trainium2 NeuronCores (Bass/Tile), data-parallel over batch.

Problem: B,L,H,E = 8,1024,8,64
  score  = (1/sqrt(E)) * einsum('blhe,bshe->bhls', Q, K)
  gauss  = kappa/sig_l * exp(-(l-s)^2 / (2 sig_l^2))       (kappa = 1/sqrt(2 pi))
  G_V    = softmax(score, s) @ G_values
  L_V    = softmax(score + gauss, s) @ L_values

Device strategy (per core = one batch element, loop over 8 heads):
  Work in the transposed layout P[s, l] so the attention@V matmuls need no
  transposes.  With c_l = gauss[l,l] (peak), exp(score+gauss-c_l) =
  P * exp(gauss-c_l) where P = exp(score) is shared with the Global path and
  exp(gauss-c_l) equals the constant encg_l = exp(-c_l) except in a +-32 band
  around the diagonal.  So:
    L_num   = encg_l * (Vl^T P) + Vl^T (P .* W)     with W = exp(gauss-c)-encg
    L_den   = encg_l * Z + ones^T (P .* W),  Z = ones^T P
  The device computes  Vg^T P | Vl^T P | Vl^T (P.*W)  (each with a ones column
  appended to V so the Z rows come for free) and the host does the cheap
  per-column recombination/normalization.  W (banded, [128x192] per s-chunk)
  and the transposed Q^T/K^T layouts are precomputed on the host.
"""

import math
import numpy as np
import ml_dtypes

BF16 = ml_dtypes.bfloat16
B, L, H, E = 8, 1024, 8, 64
NCH = L // 128          # 8 s-chunks of 128
BAND = 32               # gauss band halfwidth (exp(-d^2/2sig^2) < 1e-55 beyond)
WW = 128 + 2 * BAND     # 192: W tile width in l per s-chunk
N_CORES = 8

_NC_CACHE = {}


def _build_nc():
    if "nc" in _NC_CACHE:
        return _NC_CACHE["nc"]
    import concourse.bacc as bacc
    import concourse.tile as tile
    from concourse import mybir
    from concourse.tile import add_dep_helper

    f32 = mybir.dt.float32
    bf16 = mybir.dt.bfloat16

    nc = bacc.Bacc()
    qkt_d = nc.declare_dram_parameter("qkt", [4, 128, 2 * L], bf16, isOutput=False)
    # vgl[h, :, k, 0:64] = V_g chunk, 64:128 = V_l chunk, col 128 = ones
    vgl_d = nc.declare_dram_parameter("vgl", [H, 128, NCH, 129], bf16, isOutput=False)
    wbd_d = nc.declare_dram_parameter("wband", [H, 128, NCH, WW], bf16, isOutput=False)
    # out[h, half, 0] = [128, 512]: rows 0:64 Vg^T P, rows 64:128 Vl^T P
    # out[h, half, 1] = [65, 512]: band correction (+ band Z row 64)
    # out[h, half, 2] = [4, 512]: Z partial sums (4 col-group streams)
    outGL_d = nc.declare_dram_parameter("outGL", [H, 2, 128, 512], f32, isOutput=True)
    outB_d = nc.declare_dram_parameter("outB", [H, 2, 65, 512], f32, isOutput=True)
    outZ_d = nc.declare_dram_parameter("outZ", [H, 2, 4, 512], f32, isOutput=True)

    with tile.TileContext(nc) as tc:
        with (
            tc.tile_pool(name="ones_p", bufs=1) as ones_p,
            tc.tile_pool(name="qkt_p", bufs=2) as qkt_p,
            tc.tile_pool(name="v_p", bufs=4) as v_p,
            tc.tile_pool(name="w_p", bufs=3) as w_p,
            tc.tile_pool(name="pg_p", bufs=20) as pg_p,
            tc.tile_pool(name="mb_p", bufs=18) as mb_p,
            tc.tile_pool(name="stg_p", bufs=3) as stg_p,
            tc.tile_pool(name="sc_p", bufs=2, space="PSUM") as sc_p,
            tc.tile_pool(name="acc_p", bufs=1, space="PSUM") as acc_p,
        ):
            ones = ones_p.tile([128, 1], bf16, tag="ones", bufs=1)
            nc.vector.memset(ones, 1.0)
            zrow = ones_p.tile([1, 512], bf16, tag="zrow", bufs=1)
            nc.vector.memset(zrow, 0.0)
            for i in range(4):  # head pairs
                qt = qkt_p.tile([128, 2 * L], bf16, tag="qkt", bufs=2)
                nc.sync.dma_start(out=qt, in_=qkt_d.ap()[i])
                vgls, wbs = [], []
                for p in range(2):
                    h = 2 * i + p
                    vgl = v_p.tile([128, NCH, 129], bf16, tag=f"vgl{p}", bufs=2)
                    nc.sync.dma_start(out=vgl, in_=vgl_d.ap()[h])
                    wb = w_p.tile([128, NCH, WW], bf16, tag=f"wb{p}", bufs=2)
                    nc.sync.dma_start(out=wb, in_=wbd_d.ap()[h])
                    vgls.append(vgl); wbs.append(wb)

                # interleaved QK: the two heads use disjoint PE row groups
                # (partitions 0:64 / 64:128) so their matmuls run concurrently.
                pg = [[], []]
                mb = [[], []]
                for k in range(NCH):
                    scs = []
                    for p in range(2):
                        pslc = slice(64 * p, 64 * p + 64)
                        sc = sc_p.tile([128, L], f32, tag=f"sc{p}", bufs=1)
                        lhsT = qt[pslc, L + 128 * k:L + 128 * (k + 1)]
                        nc.tensor.matmul(out=sc[:, 0:512], lhsT=lhsT,
                                         rhs=qt[pslc, 0:512], start=True, stop=True)
                        nc.tensor.matmul(out=sc[:, 512:1024], lhsT=lhsT,
                                         rhs=qt[pslc, 512:1024], start=True, stop=True)
                        scs.append(sc)
                    for p in range(2):
                        pgk = pg_p.tile([128, L], bf16, tag="pg", bufs=20)
                        nc.scalar.activation(
                            out=pgk, in_=scs[p],
                            func=mybir.ActivationFunctionType.Exp,
                            scale=1.0 / math.sqrt(E),
                        )
                        pg[p].append(pgk)
                        a0 = max(0, 128 * k - BAND)
                        b0 = min(L, 128 * k + 128 + BAND)
                        woff = a0 - (128 * k - BAND)
                        mbk = mb_p.tile([128, WW], bf16, tag="mb", bufs=18)
                        nc.vector.tensor_mul(
                            out=mbk[:, woff:woff + (b0 - a0)],
                            in0=pgk[:, a0:b0],
                            in1=wbs[p][:, k, woff:woff + (b0 - a0)],
                        )
                        mb[p].append(mbk)

                for p in range(2):
                    h = 2 * i + p
                    vgl = vgls[p]
                    for half in range(2):
                        h0 = half * 512
                        accGL = acc_p.tile([128, 512], f32, tag="accGL", bufs=1)
                        accB = acc_p.tile([65, 512], f32, tag="accB", bufs=1)
                        accZ = acc_p.tile([128, 512], f32, tag="accZ", bufs=1)
                        # Z bank: one K=1 zero-matmul writes the whole bank
                        # (start=True -> clears + sets has_written everywhere),
                        # then the 4 col-group-packed ones-streams all
                        # accumulate with start=False -- no mid-stream bank
                        # clears, so concurrent execution is safe.
                        z_clear = nc.tensor.matmul(out=accZ, lhsT=zrow[:, 0:128],
                                                   rhs=zrow, start=True, stop=False,
                                                   skip_group_check=True)
                        for k in range(NCH):
                            rhs = pg[p][k][:, h0:h0 + 512]
                            # G|L fused: stationary [V_g | V_l], M=128
                            nc.tensor.matmul(out=accGL, lhsT=vgl[:, k, 0:128],
                                             rhs=rhs,
                                             start=(k == 0), stop=(k == NCH - 1))
                            j = k % 4
                            mmz = nc.tensor.matmul(out=accZ[32 * j:32 * j + 1, :],
                                                   lhsT=ones, rhs=rhs,
                                                   start=False, stop=(k >= 4),
                                                   tile_position=(0, 32 * j),
                                                   skip_group_check=True)
                            add_dep_helper(mmz.ins, z_clear.ins,
                                           reason="z bank clear first")
                        spans = []
                        for k in range(NCH):
                            a = max(0, 128 * k - BAND, h0)
                            bb = min(L, 128 * k + 128 + BAND, h0 + 512)
                            if bb > a:
                                spans.append((k, a, bb))
                        b_first = None
                        for j, (k, a, bb) in enumerate(spans):
                            off = a - (128 * k - BAND)
                            mmb = nc.tensor.matmul(
                                out=accB[:, a - h0:bb - h0],
                                lhsT=vgl[:, k, 64:129],
                                rhs=mb[p][k][:, off:off + (bb - a)],
                                start=(j == 0), stop=(j == len(spans) - 1),
                                skip_group_check=True,
                            )
                            if j == 0:
                                b_first = mmb
                            else:
                                add_dep_helper(mmb.ins, b_first.ins,
                                               reason="bank clear first")
                        stgGL = stg_p.tile([128, 512], f32, tag="stgGL", bufs=3)
                        nc.vector.tensor_copy(out=stgGL, in_=accGL)
                        nc.sync.dma_start(out=outGL_d.ap()[h, half], in_=stgGL)
                        stgB = stg_p.tile([65, 512], f32, tag="stgB", bufs=3)
                        nc.vector.tensor_copy(out=stgB, in_=accB)
                        nc.sync.dma_start(out=outB_d.ap()[h, half], in_=stgB)
                        stgZ = stg_p.tile([128, 512], f32, tag="stgZ", bufs=3)
                        nc.vector.tensor_copy(out=stgZ, in_=accZ)
                        nc.sync.dma_start(out=outZ_d.ap()[h, half],
                                          in_=stgZ[0:128:32, :])
    nc.compile()
    _NC_CACHE["nc"] = nc
    return nc


def _host_prep(G_queries, G_keys, G_values, L_values, sigma):
    """Build per-core input dicts + host-side encg [L, H] per core."""
    inv_sqrt_2pi = 1.0 / math.sqrt(2.0 * math.pi)
    sig = sigma.astype(np.float32)
    sig = 1.0 / (1.0 + np.exp(-5.0 * sig.astype(np.float64)))
    sig = (sig + 1e-05).astype(np.float32)
    sig = (np.float32(3.0) ** sig) - np.float32(1.0)          # [B, L, H]
    c = inv_sqrt_2pi / sig.astype(np.float64)                  # [B, L, H]
    encg = np.exp(-c)                                          # [B, L, H]
    nhi = 1.0 / (2.0 * sig.astype(np.float64) ** 2)

    in_maps = []
    aux = []
    for b in range(B):
        qkt = np.empty((4, 128, 2 * L), BF16)
        for h in range(H):
            i, p = divmod(h, 2)
            qkt[i, 64 * p:64 * p + 64, :L] = G_queries[b, :, h, :].T
            qkt[i, 64 * p:64 * p + 64, L:] = G_keys[b, :, h, :].T
        # packed V: [V_g | V_l | ones] per chunk, layout [H, 128, NCH, 129]
        vgl = np.empty((H, 128, NCH, 129), BF16)
        gv = G_values[b].reshape(NCH, 128, H, E)   # [k, p, h, e]
        lv = L_values[b].reshape(NCH, 128, H, E)
        vgl[:, :, :, 0:64] = np.ascontiguousarray(gv.transpose(2, 1, 0, 3))
        vgl[:, :, :, 64:128] = np.ascontiguousarray(lv.transpose(2, 1, 0, 3))
        vgl[..., 128] = 1.0
        # W band tiles [H, 128, NCH, WW]
        wband = np.zeros((H, 128, NCH, WW), BF16)
        s_off = np.arange(128)
        j_off = np.arange(WW)
        for k in range(NCH):
            s_idx = 128 * k + s_off                  # [128]
            l_idx = 128 * k - BAND + j_off           # [WW]
            valid = (l_idx >= 0) & (l_idx < L)
            lvx = np.clip(l_idx, 0, L - 1)
            d = l_idx[None, :] - s_idx[:, None]      # [128, WW]
            band_ok = (np.abs(d) <= BAND) & valid[None, :]
            for h in range(H):
                ch = c[b, lvx, h][None, :]
                g = ch * np.exp(-(d.astype(np.float64) ** 2) * nhi[b, lvx, h][None, :])
                W = np.exp(g - ch) - encg[b, lvx, h][None, :]
                W[~band_ok] = 0.0
                wband[h, :, k, :] = W.astype(np.float32)
        in_maps.append({"qkt": np.asarray(qkt),
                        "vgl": np.asarray(vgl),
                        "wband": np.asarray(wband)})
        aux.append(encg[b])  # [L, H]
    return in_maps, aux


def _host_post(outs, aux):
    G_V = np.empty((B, L, H, E), np.float32)
    L_V = np.empty((B, L, H, E), np.float32)
    for b in range(B):
        oGL = outs[b]["outGL"].astype(np.float64)  # [H, 2, 128, 512]
        oB = outs[b]["outB"].astype(np.float64)    # [H, 2, 65, 512]
        oZ = outs[b]["outZ"].astype(np.float64)    # [H, 2, 4, 512]
        for h in range(H):
            GLt = np.concatenate([oGL[h, 0], oGL[h, 1]], axis=1)  # [128, L]
            Bt = np.concatenate([oB[h, 0], oB[h, 1]], axis=1)     # [65, L]
            Z = np.concatenate([oZ[h, 0], oZ[h, 1]], axis=1).sum(axis=0)  # [L]
            e = aux[b][:, h]  # [L]
            G_V[b, :, h, :] = (GLt[0:64] / Z).T
            Lnum = GLt[64:128] * e[None, :] + Bt[:64]
            Lden = Z * e + Bt[64]
            L_V[b, :, h, :] = (Lnum / Lden).T
    return G_V, L_V


def kernel(G_queries, G_keys, G_values, L_values, sigma):
    from concourse.bass_utils import run_bass_kernel_spmd

    args = [np.asarray(x, dtype=np.float32) for x in
            (G_queries, G_keys, G_values, L_values, sigma)]
    nc = _build_nc()
    in_maps, aux = _host_prep(*args)
    res = run_bass_kernel_spmd(nc, in_maps, core_ids=list(range(N_CORES)),
                               trace=False)
    return _host_post(res.results, aux)


# revision 17
# speedup vs baseline: 1.1657x; 1.1657x over previous
"""AnomalyAttention on 8 Trainium2 NeuronCores (Bass/Tile), data-parallel over batch.

Problem: B,L,H,E = 8,1024,8,64
  score  = (1/sqrt(E)) * einsum('blhe,bshe->bhls', Q, K)
  gauss  = kappa/sig_l * exp(-(l-s)^2 / (2 sig_l^2))       (kappa = 1/sqrt(2 pi))
  G_V    = softmax(score, s) @ G_values
  L_V    = softmax(score + gauss, s) @ L_values

Device strategy (per core = one batch element, loop over 8 heads):
  Work in the transposed layout P[s, l] so the attention@V matmuls need no
  transposes.  With c_l = gauss[l,l] (peak), exp(score+gauss-c_l) =
  P * exp(gauss-c_l) where P = exp(score) is shared with the Global path and
  exp(gauss-c_l) equals the constant encg_l = exp(-c_l) except in a +-32 band
  around the diagonal.  So:
    L_num   = encg_l * (Vl^T P) + Vl^T (P .* W)     with W = exp(gauss-c)-encg
    L_den   = encg_l * Z + ones^T (P .* W),  Z = ones^T P
  The device computes  Vg^T P | Vl^T P | Vl^T (P.*W)  (each with a ones column
  appended to V so the Z rows come for free) and the host does the cheap
  per-column recombination/normalization.  W (banded, [128x192] per s-chunk)
  and the transposed Q^T/K^T layouts are precomputed on the host.
"""

import math
import numpy as np
import ml_dtypes

BF16 = ml_dtypes.bfloat16
B, L, H, E = 8, 1024, 8, 64
NCH = L // 128          # 8 s-chunks of 128
BAND = 32               # gauss band halfwidth (exp(-d^2/2sig^2) < 1e-55 beyond)
WW = 128 + 2 * BAND     # 192: W tile width in l per s-chunk
N_CORES = 8

_NC_CACHE = {}


def _build_nc():
    if "nc" in _NC_CACHE:
        return _NC_CACHE["nc"]
    import concourse.bacc as bacc
    import concourse.tile as tile
    from concourse import mybir
    from concourse.tile import add_dep_helper

    f32 = mybir.dt.float32
    bf16 = mybir.dt.bfloat16

    nc = bacc.Bacc()
    qkt_d = nc.declare_dram_parameter("qkt", [4, 128, 2 * L], bf16, isOutput=False)
    # vgl[h, :, k, 0:64] = V_g chunk, 64:128 = V_l chunk, col 128 = ones
    vgl_d = nc.declare_dram_parameter("vgl", [H, 128, NCH, 129], bf16, isOutput=False)
    wbd_d = nc.declare_dram_parameter("wband", [H, 128, NCH, WW], bf16, isOutput=False)
    # out[h, half, 0] = [128, 512]: rows 0:64 Vg^T P, rows 64:128 Vl^T P
    # out[h, half, 1] = [65, 512]: band correction (+ band Z row 64)
    # out[h, half, 2] = [4, 512]: Z partial sums (4 col-group streams)
    outGL_d = nc.declare_dram_parameter("outGL", [H, 2, 128, 512], f32, isOutput=True)
    outB_d = nc.declare_dram_parameter("outB", [H, 2, 65, 512], f32, isOutput=True)
    outZ_d = nc.declare_dram_parameter("outZ", [H, 2, 4, 512], f32, isOutput=True)

    with tile.TileContext(nc) as tc:
        with (
            tc.tile_pool(name="ones_p", bufs=1) as ones_p,
            tc.tile_pool(name="qkt_p", bufs=2) as qkt_p,
            tc.tile_pool(name="v_p", bufs=4) as v_p,
            tc.tile_pool(name="w_p", bufs=3) as w_p,
            tc.tile_pool(name="pg_p", bufs=20) as pg_p,
            tc.tile_pool(name="mb_p", bufs=18) as mb_p,
            tc.tile_pool(name="stg_p", bufs=3) as stg_p,
            tc.tile_pool(name="sc_p", bufs=2, space="PSUM") as sc_p,
            tc.tile_pool(name="acc_p", bufs=1, space="PSUM") as acc_p,
        ):
            ones = ones_p.tile([128, 1], bf16, tag="ones", bufs=1)
            nc.vector.memset(ones, 1.0)
            zrow = ones_p.tile([1, 512], bf16, tag="zrow", bufs=1)
            nc.vector.memset(zrow, 0.0)
            for i in range(4):  # head pairs
                qt = qkt_p.tile([128, 2 * L], bf16, tag="qkt", bufs=2)
                nc.sync.dma_start(out=qt, in_=qkt_d.ap()[i])
                vgls, wbs = [], []
                for p in range(2):
                    h = 2 * i + p
                    vgl = v_p.tile([128, NCH, 129], bf16, tag=f"vgl{p}", bufs=2)
                    nc.sync.dma_start(out=vgl, in_=vgl_d.ap()[h])
                    wb = w_p.tile([128, NCH, WW], bf16, tag=f"wb{p}", bufs=2)
                    nc.sync.dma_start(out=wb, in_=wbd_d.ap()[h])
                    vgls.append(vgl); wbs.append(wb)

                # interleaved QK: the two heads use disjoint PE row groups
                # (partitions 0:64 / 64:128) so their matmuls run concurrently.
                pg = [[], []]
                mb = [[], []]
                for k in range(NCH):
                    scs = []
                    for p in range(2):
                        pslc = slice(64 * p, 64 * p + 64)
                        sc = sc_p.tile([128, L], f32, tag=f"sc{p}", bufs=1)
                        lhsT = qt[pslc, L + 128 * k:L + 128 * (k + 1)]
                        nc.tensor.matmul(out=sc[:, 0:512], lhsT=lhsT,
                                         rhs=qt[pslc, 0:512], start=True, stop=True)
                        nc.tensor.matmul(out=sc[:, 512:1024], lhsT=lhsT,
                                         rhs=qt[pslc, 512:1024], start=True, stop=True)
                        scs.append(sc)
                    for p in range(2):
                        pgk = pg_p.tile([128, L], bf16, tag="pg", bufs=20)
                        nc.scalar.activation(
                            out=pgk, in_=scs[p],
                            func=mybir.ActivationFunctionType.Exp,
                            scale=1.0 / math.sqrt(E),
                        )
                        pg[p].append(pgk)
                        a0 = max(0, 128 * k - BAND)
                        b0 = min(L, 128 * k + 128 + BAND)
                        woff = a0 - (128 * k - BAND)
                        mbk = mb_p.tile([128, WW], bf16, tag="mb", bufs=18)
                        nc.vector.tensor_mul(
                            out=mbk[:, woff:woff + (b0 - a0)],
                            in0=pgk[:, a0:b0],
                            in1=wbs[p][:, k, woff:woff + (b0 - a0)],
                        )
                        mb[p].append(mbk)

                for p in range(2):
                    h = 2 * i + p
                    vgl = vgls[p]
                    for half in range(2):
                        h0 = half * 512
                        accGL = acc_p.tile([128, 512], f32, tag="accGL", bufs=1)
                        accB = acc_p.tile([65, 512], f32, tag="accB", bufs=1)
                        accZ = acc_p.tile([128, 512], f32, tag="accZ", bufs=1)
                        # Z bank: one K=1 zero-matmul writes the whole bank
                        # (start=True -> clears + sets has_written everywhere),
                        # then the 4 col-group-packed ones-streams all
                        # accumulate with start=False -- no mid-stream bank
                        # clears, so concurrent execution is safe.
                        z_clear = nc.tensor.matmul(out=accZ, lhsT=zrow[:, 0:128],
                                                   rhs=zrow, start=True, stop=False,
                                                   skip_group_check=True)
                        for k in range(NCH):
                            rhs = pg[p][k][:, h0:h0 + 512]
                            # G|L fused: stationary [V_g | V_l], M=128
                            nc.tensor.matmul(out=accGL, lhsT=vgl[:, k, 0:128],
                                             rhs=rhs,
                                             start=(k == 0), stop=(k == NCH - 1))
                        # Z ones-matmuls: emitted back-to-back in rounds of 4
                        # col groups so they execute concurrently on the PE.
                        for k in range(NCH):
                            rhs = pg[p][k][:, h0:h0 + 512]
                            j = k % 4
                            mmz = nc.tensor.matmul(out=accZ[32 * j:32 * j + 1, :],
                                                   lhsT=ones, rhs=rhs,
                                                   start=False, stop=(k >= 4),
                                                   tile_position=(0, 32 * j),
                                                   skip_group_check=True)
                            add_dep_helper(mmz.ins, z_clear.ins,
                                           reason="z bank clear first")
                        spans = []
                        for k in range(NCH):
                            a = max(0, 128 * k - BAND, h0)
                            bb = min(L, 128 * k + 128 + BAND, h0 + 512)
                            if bb > a:
                                spans.append((k, a, bb))
                        b_first = None
                        for j, (k, a, bb) in enumerate(spans):
                            off = a - (128 * k - BAND)
                            mmb = nc.tensor.matmul(
                                out=accB[:, a - h0:bb - h0],
                                lhsT=vgl[:, k, 64:129],
                                rhs=mb[p][k][:, off:off + (bb - a)],
                                start=(j == 0), stop=(j == len(spans) - 1),
                                skip_group_check=True,
                            )
                            if j == 0:
                                b_first = mmb
                            else:
                                add_dep_helper(mmb.ins, b_first.ins,
                                               reason="bank clear first")
                        stgGL = stg_p.tile([128, 512], f32, tag="stgGL", bufs=3)
                        nc.vector.tensor_copy(out=stgGL, in_=accGL)
                        nc.sync.dma_start(out=outGL_d.ap()[h, half], in_=stgGL)
                        stgB = stg_p.tile([65, 512], f32, tag="stgB", bufs=3)
                        nc.vector.tensor_copy(out=stgB, in_=accB)
                        nc.sync.dma_start(out=outB_d.ap()[h, half], in_=stgB)
                        stgZ = stg_p.tile([128, 512], f32, tag="stgZ", bufs=3)
                        nc.vector.tensor_copy(out=stgZ, in_=accZ)
                        nc.sync.dma_start(out=outZ_d.ap()[h, half],
                                          in_=stgZ[0:128:32, :])
    nc.compile()
    _NC_CACHE["nc"] = nc
    return nc


def _host_prep(G_queries, G_keys, G_values, L_values, sigma):
    """Build per-core input dicts + host-side encg [L, H] per core."""
    inv_sqrt_2pi = 1.0 / math.sqrt(2.0 * math.pi)
    sig = sigma.astype(np.float32)
    sig = 1.0 / (1.0 + np.exp(-5.0 * sig.astype(np.float64)))
    sig = (sig + 1e-05).astype(np.float32)
    sig = (np.float32(3.0) ** sig) - np.float32(1.0)          # [B, L, H]
    c = inv_sqrt_2pi / sig.astype(np.float64)                  # [B, L, H]
    encg = np.exp(-c)                                          # [B, L, H]
    nhi = 1.0 / (2.0 * sig.astype(np.float64) ** 2)

    in_maps = []
    aux = []
    for b in range(B):
        qkt = np.empty((4, 128, 2 * L), BF16)
        for h in range(H):
            i, p = divmod(h, 2)
            qkt[i, 64 * p:64 * p + 64, :L] = G_queries[b, :, h, :].T
            qkt[i, 64 * p:64 * p + 64, L:] = G_keys[b, :, h, :].T
        # packed V: [V_g | V_l | ones] per chunk, layout [H, 128, NCH, 129]
        vgl = np.empty((H, 128, NCH, 129), BF16)
        gv = G_values[b].reshape(NCH, 128, H, E)   # [k, p, h, e]
        lv = L_values[b].reshape(NCH, 128, H, E)
        vgl[:, :, :, 0:64] = np.ascontiguousarray(gv.transpose(2, 1, 0, 3))
        vgl[:, :, :, 64:128] = np.ascontiguousarray(lv.transpose(2, 1, 0, 3))
        vgl[..., 128] = 1.0
        # W band tiles [H, 128, NCH, WW]
        wband = np.zeros((H, 128, NCH, WW), BF16)
        s_off = np.arange(128)
        j_off = np.arange(WW)
        for k in range(NCH):
            s_idx = 128 * k + s_off                  # [128]
            l_idx = 128 * k - BAND + j_off           # [WW]
            valid = (l_idx >= 0) & (l_idx < L)
            lvx = np.clip(l_idx, 0, L - 1)
            d = l_idx[None, :] - s_idx[:, None]      # [128, WW]
            band_ok = (np.abs(d) <= BAND) & valid[None, :]
            for h in range(H):
                ch = c[b, lvx, h][None, :]
                g = ch * np.exp(-(d.astype(np.float64) ** 2) * nhi[b, lvx, h][None, :])
                W = np.exp(g - ch) - encg[b, lvx, h][None, :]
                W[~band_ok] = 0.0
                wband[h, :, k, :] = W.astype(np.float32)
        in_maps.append({"qkt": np.asarray(qkt),
                        "vgl": np.asarray(vgl),
                        "wband": np.asarray(wband)})
        aux.append(encg[b])  # [L, H]
    return in_maps, aux


def _host_post(outs, aux):
    G_V = np.empty((B, L, H, E), np.float32)
    L_V = np.empty((B, L, H, E), np.float32)
    for b in range(B):
        oGL = outs[b]["outGL"].astype(np.float64)  # [H, 2, 128, 512]
        oB = outs[b]["outB"].astype(np.float64)    # [H, 2, 65, 512]
        oZ = outs[b]["outZ"].astype(np.float64)    # [H, 2, 4, 512]
        for h in range(H):
            GLt = np.concatenate([oGL[h, 0], oGL[h, 1]], axis=1)  # [128, L]
            Bt = np.concatenate([oB[h, 0], oB[h, 1]], axis=1)     # [65, L]
            Z = np.concatenate([oZ[h, 0], oZ[h, 1]], axis=1).sum(axis=0)  # [L]
            e = aux[b][:, h]  # [L]
            G_V[b, :, h, :] = (GLt[0:64] / Z).T
            Lnum = GLt[64:128] * e[None, :] + Bt[:64]
            Lden = Z * e + Bt[64]
            L_V[b, :, h, :] = (Lnum / Lden).T
    return G_V, L_V


def kernel(G_queries, G_keys, G_values, L_values, sigma):
    from concourse.bass_utils import run_bass_kernel_spmd

    args = [np.asarray(x, dtype=np.float32) for x in
            (G_queries, G_keys, G_values, L_values, sigma)]
    nc = _build_nc()
    in_maps, aux = _host_prep(*args)
    res = run_bass_kernel_spmd(nc, in_maps, core_ids=list(range(N_CORES)),
                               trace=False)
    return _host_post(res.results, aux)
